# revision 1
# baseline (speedup 1.0000x reference)
"""YOLOv3-style detection decode kernel for Trainium2 (8 NeuronCores).

Data-parallel over the batch dim (32 batches -> 4 per core). Per core:
  - class scores: PE-transpose [channels, cells] blocks into [cells, ch] PSUM
    tiles, copy to SBUF, hardware max8/max_index for per-cell argmax over the
    80 classes (first-occurrence ties, matching jnp.argmax).
  - box attrs: chunked plane layout [(b,a,chunk), cells], elementwise decode
    on Vector/Scalar engines.
  - outputs are written planar per quantity; the host interleaves to [N, 6].
"""

import sys

import numpy as np

if "/opt/trn_rl_repo" not in sys.path:
    sys.path.insert(0, "/opt/trn_rl_repo")

NUM_ATTRS = 85
B_LOC = 4  # batches per core (32 / 8)
N_CORES = 8

# (name, H, stride, chunks)
_SCALES = (
    ("13", 13, 32.0, 2),
    ("26", 26, 16.0, 6),
    ("52", 52, 8.0, 8),
)


def _scale_cfg():
    cfgs = []
    for name, H, stride, c in _SCALES:
        HW = H * H
        nblk = -(-HW // 128)  # active 128-cell blocks
        HWp = 128 * ((c * ((HW + c - 1) // c) + 127) // 128)
        # chunked plane layout: P = B_LOC*3*c partitions, Fp cells per chunk
        # require c*Fp == HWp and Fp arbitrary; pick HWp = multiple of 128*? so
        # that Fp = HWp // c is integral.
        while HWp % c:
            HWp += 128
        Fp = HWp // c
        P = B_LOC * 3 * c
        nblk_pad = HWp // 128
        cfgs.append(
            dict(name=name, H=H, W=H, HW=HW, stride=stride, c=c, Fp=Fp, P=P,
                 HWp=HWp, nblk=nblk_pad, nact=nblk)
        )
    return cfgs


SCFG = _scale_cfg()


def _build_program():
    import concourse.bass as bass
    import concourse.mybir as mybir
    from concourse.tile import TileContext

    f32 = mybir.dt.float32
    u32 = mybir.dt.uint32
    Alu = mybir.AluOpType
    Act = mybir.ActivationFunctionType

    nc = bass.Bass(trn_type="TRN2")

    # ---- DRAM parameters ----
    xin = {}
    pl = {}
    gx = {}
    gy = {}
    aw = {}
    ah = {}
    obox = {}
    ocls = {}
    oconf = {}
    for s in SCFG:
        n = s["name"]
        xin[n] = nc.declare_dram_parameter(f"x{n}", [B_LOC, 255, s["HW"]], f32, False)
        pl[n] = nc.declare_dram_parameter(f"pl{n}", [s["P"], 5 * s["Fp"]], f32, False)
        # packed consts: [gx (Fp) | gy (Fp) | aw (1) | ah (1)]
        gx[n] = nc.declare_dram_parameter(
            f"cst{n}", [s["P"], 2 * s["Fp"] + 2], f32, False)
        obox[n] = nc.declare_dram_parameter(f"obox{n}", [s["P"], 4, s["Fp"]], f32, True)
        ocls[n] = nc.declare_dram_parameter(
            f"ocls{n}", [B_LOC, 3 * s["nblk"], 128], f32, True)
        oconf[n] = nc.declare_dram_parameter(
            f"oconf{n}", [B_LOC, 3 * s["nblk"], 128], f32, True)
    thr_p = nc.declare_dram_parameter("thr", [128, 1], f32, False)
    idn_p = nc.declare_dram_parameter("idn", [128, 128], f32, False)

    with TileContext(nc) as tc:
        from contextlib import ExitStack
        with ExitStack() as ctx:
            cpool = ctx.enter_context(tc.tile_pool(name="consts", bufs=1))
            inpool = ctx.enter_context(tc.tile_pool(name="in", bufs=2))
            plpool = ctx.enter_context(tc.tile_pool(name="plane", bufs=2))
            bpool = ctx.enter_context(tc.tile_pool(name="batched", bufs=2))
            pspool = ctx.enter_context(
                tc.tile_pool(name="psum", bufs=1, space="PSUM"))

            thr_dma = cpool.tile([128, 1], f32, tag="thr_dma")
            nc.sync.dma_start(out=thr_dma[:, :], in_=thr_p[:, :])
            idn_t = cpool.tile([128, 128], f32, tag="idn")
            nc.sync.dma_start(out=idn_t[:, :], in_=idn_p[:, :])
            # stage threshold through the DVE so TensorScalarPtr ops never
            # need a DMA wait for their scalar (TS-ptr has 1 wait slot).
            thr_t = cpool.tile([128, 1], f32, tag="thr")
            nc.vector.tensor_copy(out=thr_t[:, :], in_=thr_dma[:, :])

            # Long-lived PSUM supertiles (3 banks each) + output transpose
            # banks. The transpose matmuls' LDWEIGHTS only tolerate ONE sync
            # wait, so the structure below keeps every matmul at <=1
            # unobserved dependency (tiny PE "observer" transposes absorb DMA
            # and WAR waits one at a time).
            GRP = 6  # blocks per supertile (6 * 256 cols * 4B = 3 banks)
            st_a = pspool.tile([128, 256 * GRP], f32, tag="sa")
            st_b = pspool.tile([128, 256 * GRP], f32, tag="sb")
            st_ps = [st_a, st_b]
            pout_c = pspool.tile([128, 128], f32, tag="pout_c")
            pout_k = pspool.tile([128, 128], f32, tag="pout_k")
            # PE observes the identity DMA once before anything else.
            nc.tensor.transpose(out=st_ps[0][0:4, 0:32],
                                in_=idn_t[0:32, 0:4],
                                identity=idn_t[0:32, 0:32])
            # initialize all PSUM + staging SBUF so bulk copies never read
            # uninitialized memory (CoreSim checks this).
            vt_a = cpool.tile([128, 256 * GRP], f32, tag="vta")
            vt_b = cpool.tile([128, 256 * GRP], f32, tag="vtb")
            st_vt = [vt_a, vt_b]
            confp = cpool.tile([128, 128], f32, tag="confp")
            clsp = cpool.tile([128, 128], f32, tag="clsp")
            # supertiles + their SBUF mirrors are initialized on ACT (their
            # steady-state writer/WAR engine); pout/confp/clsp/hb on DVE.
            for t in st_ps:
                nc.scalar.memzero(t[:, :])
            for t in st_vt:
                nc.scalar.memzero(t[:, :])
            nc.vector.memset(pout_c[:, :], 0.0)
            nc.vector.memset(pout_k[:, :], 0.0)
            nc.vector.memset(confp[:, :], 0.0)
            nc.vector.memset(clsp[:, :], 0.0)
            # per-supertile DVE heartbeat: written right after the last DVE
            # reader of each V; ACT reads it before the bulk evacuation so
            # the copy itself needs only the PE wait.
            hb_a = cpool.tile([1, 1], f32, tag="hba")
            hb_b = cpool.tile([1, 1], f32, tag="hbb")
            hb = [hb_a, hb_b]
            nc.vector.memset(hb[0][:, :], 0.0)
            nc.vector.memset(hb[1][:, :], 0.0)
            acts = cpool.tile([1, 1], f32, tag="acts")

            for s in SCFG:
                n = s["name"]
                P, Fp, HW, c = s["P"], s["Fp"], s["HW"], s["c"]
                nblk, nact = s["nblk"], s["nact"]
                stride = s["stride"]

                # ---------------- plane (box) path, all 4 batches at once ---
                cstt = cpool.tile([P, 2 * Fp + 2], f32, tag=f"cst{n}")
                nc.sync.dma_start(out=cstt[:, :], in_=gx[n][:, :])
                plt = plpool.tile([P, 5 * Fp], f32, tag="plt")
                nc.sync.dma_start(out=plt[:, :], in_=pl[n][:, :])
                gxt = cstt[:, 0:Fp]
                gyt = cstt[:, Fp:2 * Fp]
                awt = cstt[:, 2 * Fp:2 * Fp + 1]
                aht = cstt[:, 2 * Fp + 1:2 * Fp + 2]
                # DVE observes the const + plane DMAs before any TS-ptr op
                stg = plpool.tile([P, 2], f32, tag="stg")
                nc.vector.tensor_copy(out=stg[:, 0:1], in_=awt)
                nc.vector.tensor_copy(out=stg[:, 1:2], in_=plt[:, 0:1])

                conf_s = plt[:, 0 * Fp:1 * Fp]
                tx_s = plt[:, 1 * Fp:2 * Fp]
                ty_s = plt[:, 2 * Fp:3 * Fp]
                twth = plt[:, 3 * Fp:5 * Fp]

                ex = plpool.tile([P, 2 * Fp], f32, tag="ex")
                nc.scalar.activation(out=ex[:, :], in_=twth, func=Act.Exp)
                wh = plpool.tile([P, 2 * Fp], f32, tag="wh")
                nc.vector.tensor_single_scalar(
                    out=wh[:, 0:Fp], in_=ex[:, 0:Fp], scalar=awt[:, :],
                    op=Alu.mult)
                nc.vector.tensor_single_scalar(
                    out=wh[:, Fp:2 * Fp], in_=ex[:, Fp:2 * Fp],
                    scalar=aht, op=Alu.mult)
                wh2 = plpool.tile([P, 2 * Fp], f32, tag="wh2")
                nc.vector.tensor_single_scalar(
                    out=wh2[:, :], in_=wh[:, :], scalar=0.5, op=Alu.mult)

                cx = plpool.tile([P, Fp], f32, tag="cx")
                nc.vector.scalar_tensor_tensor(
                    out=cx[:, :], in0=tx_s, scalar=stride, in1=gxt,
                    op0=Alu.mult, op1=Alu.add)
                cy = plpool.tile([P, Fp], f32, tag="cy")
                nc.vector.scalar_tensor_tensor(
                    out=cy[:, :], in0=ty_s, scalar=stride, in1=gyt,
                    op0=Alu.mult, op1=Alu.add)

                maskp = plpool.tile([P, Fp], f32, tag="maskp")
                nc.vector.tensor_single_scalar(
                    out=maskp[:, :], in_=conf_s, scalar=thr_t[0:P, :],
                    op=Alu.is_gt)

                res = plpool.tile([P, 4 * Fp], f32, tag="res")
                x1 = res[:, 0 * Fp:1 * Fp]
                y1 = res[:, 1 * Fp:2 * Fp]
                x2 = res[:, 2 * Fp:3 * Fp]
                y2 = res[:, 3 * Fp:4 * Fp]
                nc.vector.tensor_sub(x1, cx[:, :], wh2[:, 0:Fp])
                nc.vector.tensor_sub(y1, cy[:, :], wh2[:, Fp:2 * Fp])
                nc.vector.tensor_add(x2, x1, wh[:, 0:Fp])
                nc.vector.tensor_add(y2, y1, wh[:, Fp:2 * Fp])
                resm = plpool.tile([P, 4 * Fp], f32, tag="resm")
                for q in range(4):
                    nc.vector.tensor_mul(
                        resm[:, q * Fp:(q + 1) * Fp],
                        res[:, q * Fp:(q + 1) * Fp], maskp[:, :])
                nc.sync.dma_start(
                    out=obox[n][:, :, :],
                    in_=resm[:, :].rearrange("p (q f) -> p q f", q=4))

                # ---------------- class / argmax path -----------------------
                for b in range(B_LOC):
                    t0 = inpool.tile([128, HW], f32, tag="t0")
                    nc.sync.dma_start(out=t0[:, :], in_=xin[n][b, 0:128, :])
                    t1 = inpool.tile([127, HW], f32, tag="t1")
                    nc.sync.dma_start(out=t1[:, :], in_=xin[n][b, 128:255, :])
                    # PE absorbs supertile-0's WAR first, then the two input
                    # DMA completions, one wait per tiny observer transpose
                    # (LDW matmuls only carry one sync wait). Observer outputs
                    # land in regions the real transposes overwrite.
                    nc.tensor.transpose(
                        out=st_ps[0][0:4, 0:32], in_=st_vt[0][0:32, 0:4],
                        identity=idn_t[0:32, 0:32])
                    nc.tensor.transpose(
                        out=st_ps[0][0:4, 0:32], in_=t0[0:32, 0:4],
                        identity=idn_t[0:32, 0:32])
                    nc.tensor.transpose(
                        out=st_ps[0][0:4, 32:64], in_=t1[0:32, 0:4],
                        identity=idn_t[0:32, 0:32])

                    ixb = bpool.tile([128, 8 * 3 * nblk], u32, tag="ixb")
                    conft = bpool.tile([128, 3 * nblk], f32, tag="conft")
                    nc.gpsimd.memset(ixb[:, :], 0)
                    nc.gpsimd.memset(conft[:, :], 0.0)
                    # DVE observes the GP memsets (keeps max_index at 1 wait)
                    dstg = bpool.tile([128, 2], f32, tag="dstg")
                    nc.vector.tensor_copy(out=dstg[:, 0:1],
                                          in_=conft[:, 0:1])
                    nc.vector.tensor_copy(
                        out=dstg[:, 1:2],
                        in_=ixb[:, 0:1].bitcast(f32))

                    ngrp = -(-nact // GRP)
                    for g in range(ngrp):
                        S = st_ps[g % 2]
                        V = st_vt[g % 2]
                        blks = list(range(g * GRP, min((g + 1) * GRP, nact)))
                        if g > 0:
                            # pre-observe this supertile's WAR (the ACT copy
                            # that last read it) by reading what it wrote.
                            nc.tensor.transpose(
                                out=S[0:4, 0:32], in_=V[0:32, 0:4],
                                identity=idn_t[0:32, 0:32])
                        for nb in blks:
                            f0 = nb * 128
                            fb = min(128, HW - f0)
                            base = (nb - g * GRP) * 256
                            nc.tensor.transpose(
                                out=S[0:fb, base:base + 128],
                                in_=t0[:, f0:f0 + fb], identity=idn_t[:, :])
                            nc.tensor.transpose(
                                out=S[0:fb, base + 128:base + 255],
                                in_=t1[:, f0:f0 + fb],
                                identity=idn_t[0:127, 0:127])
                        ncols = 256 * len(blks)
                        from concourse.tile import add_dep_helper
                        iobs = nc.scalar.copy(out=acts[:, :],
                                              in_=hb[g % 2][:, :])
                        icpy = nc.scalar.copy(out=V[:, 0:ncols],
                                              in_=S[:, 0:ncols])
                        add_dep_helper(icpy.ins, iobs.ins, sync=False,
                                       reason="absorb V WAR on ACT")

                        for nb in blks:
                            fb = min(128, HW - nb * 128)
                            base = (nb - g * GRP) * 256
                            vt_conf = V[0:fb, base:base + 255].rearrange(
                                "p (a t) -> p a t", a=3)[:, :, 0]
                            cdst = conft[0:fb, :].rearrange(
                                "p (a k) -> p a k", k=nblk)[:, :, nb]
                            nc.vector.tensor_copy(out=cdst, in_=vt_conf)
                            for a in range(3):
                                cls_in = V[0:fb,
                                           base + 85 * a + 5:base + 85 * a + 85]
                                mx8 = bpool.tile([128, 8], f32, tag="mx8")
                                nc.vector.max(out=mx8[0:fb, :], in_=cls_in)
                                col = (a * nblk + nb) * 8
                                nc.vector.max_index(
                                    out=ixb[0:fb, col:col + 8],
                                    in_max=mx8[0:fb, :], in_values=cls_in)
                        nc.vector.tensor_copy(out=hb[g % 2][:, :],
                                              in_=V[0:1, 0:1])

                    # batched epilogue for (scale, batch)
                    maskt = bpool.tile([128, 3 * nblk], f32, tag="maskt")
                    nc.vector.tensor_single_scalar(
                        out=maskt[:, :], in_=conft[:, :], scalar=thr_t[:, :],
                        op=Alu.is_gt)
                    confm = bpool.tile([128, 3 * nblk], f32, tag="confm")
                    nc.vector.tensor_mul(confm[:, :], conft[:, :], maskt[:, :])
                    clsb = bpool.tile([128, 3 * nblk], f32, tag="clsb")
                    ix0 = ixb[:, :].rearrange("p (c e) -> p c e", e=8)[:, :, 0]
                    nc.vector.tensor_copy(out=clsb[:, :], in_=ix0)
                    clsm = bpool.tile([128, 3 * nblk], f32, tag="clsm")
                    nc.vector.tensor_mul(clsm[:, :], clsb[:, :], maskt[:, :])

                    # transpose back to planar [3*nblk, 128]; DVE evacuates
                    # so data-dep and WAR share the DVE semaphore.
                    nc.tensor.transpose(
                        out=pout_c[0:3 * nblk, 0:128], in_=confm[:, :],
                        identity=idn_t[:, :])
                    nc.vector.tensor_copy(out=confp[0:3 * nblk, :],
                                          in_=pout_c[0:3 * nblk, 0:128])
                    nc.sync.dma_start(out=oconf[n][b, :, :],
                                      in_=confp[0:3 * nblk, :])

                    nc.tensor.transpose(
                        out=pout_k[0:3 * nblk, 0:128], in_=clsm[:, :],
                        identity=idn_t[:, :])
                    nc.vector.tensor_copy(out=clsp[0:3 * nblk, :],
                                          in_=pout_k[0:3 * nblk, 0:128])
                    nc.sync.dma_start(out=ocls[n][b, :, :],
                                      in_=clsp[0:3 * nblk, :])

    return nc


def _split_sync_waits(nc, limit=1):
    """Move overflow sync waits onto standalone NoOps.

    walrus's codegen embeds on_wait entries into each instruction's sync
    fields and several instruction structs (LDWEIGHTS, ACTIVATE, TS-ptr)
    only have room for one; it hard-errors with "Too many sync wait
    commands" otherwise. Semantically a preceding NoOp on the same engine
    queue that carries the extra waits is equivalent.
    """
    import concourse.mybir as mybir

    for f in nc.m.functions:
        for b in f.blocks:
            insts = list(b.instructions)
            out = []
            changed = False
            for i in insts:
                si = i.sync_info
                tname = type(i).__name__
                if (si is not None and si.on_wait
                        and len(si.on_wait) > limit
                        and tname not in ("InstEventSemaphore",)):
                    waits = list(si.on_wait)
                    keep = waits[-limit:]
                    spill = waits[:-limit]
                    for k, w in enumerate(spill):
                        nop = mybir.InstNoOp(
                            name=f"{i.name}-sw{k}", ins=[], outs=[])
                        nop.engine = i.engine
                        nop.sync_info = mybir.SyncInfo(
                            on_wait=[w], on_update=[])
                        out.append(nop)
                    i.sync_info = mybir.SyncInfo(
                        on_wait=keep, on_update=list(si.on_update or []))
                    changed = True
                out.append(i)
            if changed:
                b.instructions = out


_NC_CACHE = None


def _get_program(split=True):
    global _NC_CACHE
    if _NC_CACHE is None:
        _NC_CACHE = _build_program()
    if split and not getattr(_NC_CACHE, "_waits_split", False):
        _split_sync_waits(_NC_CACHE)
        _NC_CACHE._waits_split = True
    return _NC_CACHE


def _core_inputs(core, outs, anchors, threshold):
    """Build the DRAM input map for one core. Pure data marshaling."""
    m = {}
    thrv = np.float32(threshold[0])
    for s, x_full, anch in zip(SCFG, outs, anchors):
        n = s["name"]
        HW, Fp, P, c = s["HW"], s["Fp"], s["P"], s["c"]
        HWp = s["HWp"]
        x = np.ascontiguousarray(
            x_full[core * B_LOC:(core + 1) * B_LOC].reshape(B_LOC, 255, HW),
            dtype=np.float32)
        m[f"x{n}"] = x
        # plane gather: [(b, a, ch), (attr, f)]
        idx = [a * NUM_ATTRS + t for a in range(3) for t in range(5)]
        v = x[:, idx, :].reshape(B_LOC, 3, 5, HW)
        vp = np.zeros((B_LOC, 3, 5, HWp), np.float32)
        vp[..., :HW] = v
        m[f"pl{n}"] = np.ascontiguousarray(
            vp.reshape(B_LOC, 3, 5, c, Fp).transpose(0, 1, 3, 2, 4)
            .reshape(P, 5 * Fp))
        # grids (pre-scaled by stride)
        W = s["W"]
        gxv = np.zeros(HWp, np.float32)
        gyv = np.zeros(HWp, np.float32)
        hw = np.arange(HW)
        gxv[:HW] = (hw % W) * s["stride"]
        gyv[:HW] = (hw // W) * s["stride"]
        cst = np.zeros((P, 2 * Fp + 2), np.float32)
        cst[:, 0:Fp] = np.broadcast_to(
            gxv.reshape(1, 1, c, Fp), (B_LOC, 3, c, Fp)).reshape(P, Fp)
        cst[:, Fp:2 * Fp] = np.broadcast_to(
            gyv.reshape(1, 1, c, Fp), (B_LOC, 3, c, Fp)).reshape(P, Fp)
        cst[:, 2 * Fp] = np.broadcast_to(
            anch[:, 0].astype(np.float32).reshape(1, 3, 1),
            (B_LOC, 3, c)).reshape(P)
        cst[:, 2 * Fp + 1] = np.broadcast_to(
            anch[:, 1].astype(np.float32).reshape(1, 3, 1),
            (B_LOC, 3, c)).reshape(P)
        m[f"cst{n}"] = cst
    m["thr"] = np.full((128, 1), thrv, np.float32)
    m["idn"] = np.eye(128, dtype=np.float32)
    return m


def _assemble_core(res, core):
    """Interleave one core's planar outputs into reference row order."""
    per_scale = []
    for s in SCFG:
        n = s["name"]
        HW, Fp, c = s["HW"], s["Fp"], s["c"]
        nblk = s["nblk"]
        box = (res[f"obox{n}"].reshape(B_LOC, 3, c, 4, Fp)
               .transpose(3, 0, 1, 2, 4)
               .reshape(4, B_LOC, 3, c * Fp))[..., :HW]
        cls = res[f"ocls{n}"].reshape(B_LOC, 3, nblk * 128)[..., :HW]
        conf = res[f"oconf{n}"].reshape(B_LOC, 3, nblk * 128)[..., :HW]
        block = np.stack(
            [conf, box[0], box[1], box[2], box[3], cls], axis=-1)
        # [b, a, hw, 6] -> [b, hw, a, 6]
        per_scale.append(
            block.transpose(0, 2, 1, 3).reshape(B_LOC * HW * 3, 6))
    return per_scale  # list of [B_LOC*HW*3, 6] per scale


def kernel(output_13, output_26, output_52, anchors_13, anchors_26,
           anchors_52, threshold):
    from concourse.bass_utils import run_bass_kernel_spmd

    nc = _get_program()
    outs = (np.asarray(output_13), np.asarray(output_26),
            np.asarray(output_52))
    anchors = (np.asarray(anchors_13), np.asarray(anchors_26),
               np.asarray(anchors_52))
    thr = np.asarray(threshold)

    in_maps = [_core_inputs(cc, outs, anchors, thr) for cc in range(N_CORES)]
    r = run_bass_kernel_spmd(nc, in_maps, list(range(N_CORES)))
    per_core = [_assemble_core(r.results[cc], cc) for cc in range(N_CORES)]
    blocks = []
    for si in range(3):
        blocks.append(np.concatenate([per_core[cc][si]
                                      for cc in range(N_CORES)], axis=0))
    return np.concatenate(blocks, axis=0).astype(np.float32)



# revision 11
# speedup vs baseline: 2.6533x; 2.6533x over previous
"""YOLOv3-style detection decode kernel for Trainium2 (8 NeuronCores).

Data-parallel over batch (32 -> 4 per core). Host marshals each core's head
tensors into a cells-on-partitions layout x[p, (b k a c)] (cell = k*128+p,
c = 85 attrs per anchor); since 3 anchors * 85 = 255 = the channel count,
(b, k, a) collapse into one free dim Z and the device needs no transposes:

  - argmax over the 80 classes per (cell, anchor) via two segmented DVE
    reductions: phase-maxes p8[j] = max_g x[8g+j] and group-maxes
    q10[g] = max_j x[8g+j] (one tensor_reduce each per scale). The class
    index is 8*g* + j*, with g*/j* recovered by an is_ge-against-max
    compare and a descending-weight max (ties break toward the FIRST
    index, matching jnp.argmax).
  - box decode reads strided views of the same tiles (exp/scale on ACT).
  - outputs are packed [p, b, k, a, 6] per scale; the host re-interleaves.
"""

import sys

import numpy as np

if "/opt/trn_rl_repo" not in sys.path:
    sys.path.insert(0, "/opt/trn_rl_repo")

NUM_ATTRS = 85
B_LOC = 4  # batches per core (32 / 8)
N_CORES = 8

# (name, H, stride)
_SCALES = (
    ("13", 13, 32.0),
    ("26", 26, 16.0),
    ("52", 52, 8.0),
)


def _scale_cfg():
    cfgs = []
    for name, H, stride in _SCALES:
        HW = H * H
        nblk = -(-HW // 128)
        cfgs.append(dict(name=name, H=H, W=H, HW=HW, stride=stride,
                         nblk=nblk, HWp=nblk * 128))
    return cfgs


SCFG = _scale_cfg()

# consts layout: w8(8) | w10(10) | thr(1) | per scale: gx4(4nb) gy4(4nb) anch(6)
_CST_W8 = 0
_CST_W10 = 8
_CST_THR = 18
_CST_SC = {}
_off = 19
for _s in SCFG:
    _CST_SC[_s["name"]] = _off
    _off += 8 * _s["nblk"] + 6
CST_COLS = _off


def _build_program():
    import concourse.bass as bass
    import concourse.mybir as mybir
    from concourse.tile import TileContext

    f32 = mybir.dt.float32
    Alu = mybir.AluOpType
    Act = mybir.ActivationFunctionType
    X = mybir.AxisListType.X

    nc = bass.Bass(trn_type="TRN2")

    xin = {}
    opk = {}
    for s in SCFG:
        n = s["name"]
        xin[n] = nc.declare_dram_parameter(
            f"x{n}", [128, B_LOC * s["nblk"] * 255], f32, False)
        opk[n] = nc.declare_dram_parameter(
            f"opack{n}", [128, B_LOC * s["nblk"] * 18], f32, True)
    cst_p = nc.declare_dram_parameter("cst", [128, CST_COLS], f32, False)

    with TileContext(nc) as tc:
        from contextlib import ExitStack
        with ExitStack() as ctx:
            cpool = ctx.enter_context(tc.tile_pool(name="consts", bufs=1))
            x52pool = ctx.enter_context(tc.tile_pool(name="x52", bufs=2))

            cstt = cpool.tile([128, CST_COLS], f32, tag="cst", name="cstt")
            nc.sync.dma_start(out=cstt[:, :], in_=cst_p[:, :])
            w8 = cstt[:, _CST_W8:_CST_W8 + 8]
            w10 = cstt[:, _CST_W10:_CST_W10 + 10]
            thr = cstt[:, _CST_THR:_CST_THR + 1]

            # one DVE observer of the const DMA so TS-ptr ops share one dep
            cstg = cpool.tile([128, 1], f32, tag="cstg", name="cstg")
            nc.vector.tensor_copy(out=cstg[:, :], in_=thr)

            def scale_tiles(s):
                n, nb = s["name"], s["nblk"]
                Z = B_LOC * nb * 3
                t = {}
                for key, w in (("p8", 8), ("q10", 10), ("eq8", 8),
                               ("eq10", 10), ("m", 1), ("r8", 1), ("r10", 1),
                               ("ts", 1), ("mask", 1), ("ex", 2), ("wh", 2),
                               ("cx", 1), ("cy", 1), ("x1", 1), ("y1", 1),
                               ("x2", 1), ("y2", 1), ("opk", 6)):
                    t[key] = cpool.tile([128, Z * w], f32, tag=f"{key}{n}",
                                        name=f"{key}{n}")
                return t

            def class_reduces(s, t, xtv, b):
                """Phase/group reduces + conf + cx/cy for one input view.

                xtv: [p, z, c=85] view (z spans (k a) or (b k a)); b is None
                for the all-batch case, else the batch index of a transient
                tile.
                """
                n, nb, stride = s["name"], s["nblk"], s["stride"]
                off = _CST_SC[n]
                zc = xtv.shape[1]  # cells*anchors covered by this view
                lo = 0 if b is None else b * nb * 3
                hi = lo + zc

                cls = xtv[:, :, 5:85]
                ph = cls.rearrange("p z (g j) -> p z j g", g=10, j=8)
                gr = cls.rearrange("p z (g j) -> p z g j", g=10, j=8)
                p8v = t["p8"][:, lo * 8:hi * 8].rearrange(
                    "p (z j) -> p z j", j=8)
                q10v = t["q10"][:, lo * 10:hi * 10].rearrange(
                    "p (z g) -> p z g", g=10)
                nc.vector.tensor_reduce(out=p8v, in_=ph, axis=X, op=Alu.max)
                nc.vector.tensor_reduce(out=q10v, in_=gr, axis=X, op=Alu.max)

                conf = xtv[:, :, 0:1].squeeze(2)
                maskv = t["mask"][:, lo:hi]
                nc.vector.tensor_single_scalar(
                    out=maskv, in_=conf, scalar=thr, op=Alu.is_gt)
                opkz = t["opk"][:, lo * 6:hi * 6].rearrange(
                    "p (z q) -> p z q", q=6)
                nc.vector.tensor_tensor(
                    out=opkz[:, :, 0:1].squeeze(2), in0=conf, in1=maskv,
                    op=Alu.mult)

                # exp(tw,th) on ACT; cx/cy on DVE
                exv = t["ex"][:, lo * 2:hi * 2].rearrange(
                    "p (z e) -> p z e", e=2)
                nc.scalar.activation(out=exv, in_=xtv[:, :, 3:5],
                                     func=Act.Exp)
                # gx4/gy4 are host-replicated over batches, so a [p, zc/3, 3]
                # broadcast view always lines up with this z range.
                gx = cstt[:, off + lo // 3:off + hi // 3]
                gy = cstt[:, off + 4 * nb + lo // 3:off + 4 * nb + hi // 3]
                gxb = gx.unsqueeze(2).broadcast_to([128, zc // 3, 3])
                gyb = gy.unsqueeze(2).broadcast_to([128, zc // 3, 3])
                tx = xtv[:, :, 1:2].squeeze(2).rearrange(
                    "p (w a) -> p w a", a=3)
                ty = xtv[:, :, 2:3].squeeze(2).rearrange(
                    "p (w a) -> p w a", a=3)
                cxv = t["cx"][:, lo:hi].rearrange("p (w a) -> p w a", a=3)
                cyv = t["cy"][:, lo:hi].rearrange("p (w a) -> p w a", a=3)
                nc.vector.scalar_tensor_tensor(
                    out=cxv, in0=tx, scalar=stride, in1=gxb,
                    op0=Alu.mult, op1=Alu.add)
                nc.vector.scalar_tensor_tensor(
                    out=cyv, in0=ty, scalar=stride, in1=gyb,
                    op0=Alu.mult, op1=Alu.add)

            def per_scale(s, t):
                """All-batch epilogue on compact scratch tiles."""
                n, nb = s["name"], s["nblk"]
                off = _CST_SC[n]
                anch = cstt[:, off + 8 * nb:off + 8 * nb + 6]
                Z = B_LOC * nb * 3

                def zv(tile, w):
                    return tile[:, :].rearrange("p (z q) -> p z q", q=w)

                p8 = zv(t["p8"], 8)
                q10 = zv(t["q10"], 10)
                eq8 = zv(t["eq8"], 8)
                eq10 = zv(t["eq10"], 10)
                m = t["m"][:, :]
                r8 = t["r8"][:, :]
                r10 = t["r10"][:, :]
                tsv = t["ts"][:, :]
                mask = t["mask"][:, :]
                opkq = zv(t["opk"], 6)

                nc.vector.tensor_reduce(out=m, in_=p8, axis=X, op=Alu.max)
                mb8 = m.unsqueeze(2).broadcast_to([128, Z, 8])
                mb10 = m.unsqueeze(2).broadcast_to([128, Z, 10])
                w8b = w8.unsqueeze(1).broadcast_to([128, Z, 8])
                w10b = w10.unsqueeze(1).broadcast_to([128, Z, 10])

                # j* / g* via descending-weight max (first-index tie-break)
                nc.vector.tensor_tensor(out=eq8, in0=p8, in1=mb8,
                                        op=Alu.is_ge)
                nc.vector.tensor_tensor(out=p8, in0=eq8, in1=w8b,
                                        op=Alu.mult)
                nc.vector.tensor_reduce(out=r8, in_=p8, axis=X, op=Alu.max)
                nc.vector.tensor_tensor(out=eq10, in0=q10, in1=mb10,
                                        op=Alu.is_ge)
                nc.vector.tensor_tensor(out=q10, in0=eq10, in1=w10b,
                                        op=Alu.mult)
                nc.vector.tensor_reduce(out=r10, in_=q10, axis=X, op=Alu.max)
                # idx = 88 - 8*r10 - r8 ; cls_m = (ts + 88) * mask
                nc.vector.scalar_tensor_tensor(
                    out=tsv, in0=r10, scalar=-8.0, in1=r8,
                    op0=Alu.mult, op1=Alu.subtract)
                nc.vector.scalar_tensor_tensor(
                    out=opkq[:, :, 5:6].squeeze(2), in0=tsv, scalar=88.0,
                    in1=mask, op0=Alu.add, op1=Alu.mult)

                # boxes: wh = anch * exp (ACT) -> x1/y1/x2/y2 -> masked
                exw = t["ex"][:, :].rearrange("p (w a e) -> p w a e",
                                              a=3, e=2)
                whw = t["wh"][:, :].rearrange("p (w a e) -> p w a e",
                                              a=3, e=2)
                for a in range(3):
                    for q in range(2):
                        nc.scalar.activation(
                            out=whw[:, :, a, q:q + 1].squeeze(2),
                            in_=exw[:, :, a, q:q + 1].squeeze(2),
                            func=Act.Copy,
                            scale=anch[:, 2 * a + q:2 * a + q + 1])
                whz = zv(t["wh"], 2)
                wx = whz[:, :, 0:1].squeeze(2)
                wy = whz[:, :, 1:2].squeeze(2)
                cx = t["cx"][:, :]
                cy = t["cy"][:, :]
                x1 = t["x1"][:, :]
                y1 = t["y1"][:, :]
                x2 = t["x2"][:, :]
                y2 = t["y2"][:, :]
                nc.vector.scalar_tensor_tensor(
                    out=x1, in0=wx, scalar=-0.5, in1=cx,
                    op0=Alu.mult, op1=Alu.add)
                nc.vector.scalar_tensor_tensor(
                    out=y1, in0=wy, scalar=-0.5, in1=cy,
                    op0=Alu.mult, op1=Alu.add)
                nc.vector.scalar_tensor_tensor(
                    out=x2, in0=wx, scalar=0.5, in1=cx,
                    op0=Alu.mult, op1=Alu.add)
                nc.vector.scalar_tensor_tensor(
                    out=y2, in0=wy, scalar=0.5, in1=cy,
                    op0=Alu.mult, op1=Alu.add)
                for q, src in ((1, x1), (2, y1), (3, x2), (4, y2)):
                    nc.vector.tensor_tensor(
                        out=opkq[:, :, q:q + 1].squeeze(2), in0=src,
                        in1=mask, op=Alu.mult)
                nc.sync.dma_start(out=opk[n][:, :], in_=t["opk"][:, :])

            # ---- scales 13 / 26: one static tile, all batches at once ----
            for s in SCFG[:2]:
                n = s["name"]
                t = scale_tiles(s)
                xt = cpool.tile([128, B_LOC * s["nblk"] * 255], f32,
                                tag=f"xt{n}", name=f"xt{n}")
                nc.sync.dma_start(out=xt[:, :], in_=xin[n][:, :])
                vz = xt[:, :].rearrange("p (z c) -> p z c", c=85)
                class_reduces(s, t, vz, None)
                per_scale(s, t)

            # ---- scale 52: per-batch pipelined input tiles ----
            s = SCFG[2]
            t = scale_tiles(s)
            nb = s["nblk"]
            for b in range(B_LOC):
                xtb = x52pool.tile([128, nb * 255], f32, tag="x52",
                                   name="x52b")
                nc.sync.dma_start(
                    out=xtb[:, :],
                    in_=xin[s["name"]][:, b * nb * 255:(b + 1) * nb * 255])
                vz = xtb[:, :].rearrange("p (z c) -> p z c", c=85)
                class_reduces(s, t, vz, b)
            per_scale(s, t)

    return nc


def _split_sync_waits(nc, limit=1):
    """Move overflow sync waits onto standalone NoOps (several instruction
    structs only have one wait slot; walrus hard-errors otherwise)."""
    import concourse.mybir as mybir

    for f in nc.m.functions:
        for blk in f.blocks:
            out = []
            changed = False
            for i in blk.instructions:
                si = i.sync_info
                tname = type(i).__name__
                if (si is not None and si.on_wait
                        and len(si.on_wait) > limit
                        and tname not in ("InstEventSemaphore",)):
                    waits = list(si.on_wait)
                    keep = waits[-limit:]
                    spill = waits[:-limit]
                    for k, w in enumerate(spill):
                        nop = mybir.InstNoOp(
                            name=f"{i.name}-sw{k}", ins=[], outs=[])
                        nop.engine = i.engine
                        nop.sync_info = mybir.SyncInfo(
                            on_wait=[w], on_update=[])
                        out.append(nop)
                    i.sync_info = mybir.SyncInfo(
                        on_wait=keep, on_update=list(si.on_update or []))
                    changed = True
                out.append(i)
            if changed:
                blk.instructions = out


_NC_CACHE = None


def _get_program(split=True):
    global _NC_CACHE
    if _NC_CACHE is None:
        _NC_CACHE = _build_program()
    if split and not getattr(_NC_CACHE, "_waits_split", False):
        _split_sync_waits(_NC_CACHE)
        _NC_CACHE._waits_split = True
    return _NC_CACHE


def _core_inputs(core, outs, anchors, threshold):
    """Build the DRAM input map for one core. Pure data marshaling."""
    m = {}
    for s, x_full in zip(SCFG, outs):
        n = s["name"]
        HW, nblk = s["HW"], s["nblk"]
        x = np.asarray(
            x_full[core * B_LOC:(core + 1) * B_LOC], dtype=np.float32
        ).reshape(B_LOC, 255, HW)
        xp = np.zeros((B_LOC, 255, nblk * 128), np.float32)
        xp[:, :, :HW] = x
        # [b, c, k, p] -> [p, b, k, c]
        m[f"x{n}"] = np.ascontiguousarray(
            xp.reshape(B_LOC, 255, nblk, 128).transpose(3, 0, 2, 1)
        ).reshape(128, -1)
    cst = np.zeros((128, CST_COLS), np.float32)
    cst[:, _CST_W8:_CST_W8 + 8] = (8.0 - np.arange(8))[None, :]
    cst[:, _CST_W10:_CST_W10 + 10] = (10.0 - np.arange(10))[None, :]
    cst[:, _CST_THR] = np.float32(np.asarray(threshold)[0])
    for s, anch in zip(SCFG, anchors):
        n = s["name"]
        HW, nblk, W, stride = s["HW"], s["nblk"], s["W"], s["stride"]
        off = _CST_SC[n]
        cell = (np.arange(nblk)[None, :] * 128
                + np.arange(128)[:, None])  # [p, k]
        valid = cell < HW
        gx = np.where(valid, (cell % W) * stride, 0.0).astype(np.float32)
        gy = np.where(valid, (cell // W) * stride, 0.0).astype(np.float32)
        cst[:, off:off + 4 * nblk] = np.tile(gx, (1, B_LOC))
        cst[:, off + 4 * nblk:off + 8 * nblk] = np.tile(gy, (1, B_LOC))
        cst[:, off + 8 * nblk:off + 8 * nblk + 6] = np.asarray(
            anch, np.float32).reshape(6)[None, :]
    m["cst"] = cst
    return m


def _assemble_core(res):
    """Interleave one core's packed outputs into reference row order."""
    per_scale = []
    for s in SCFG:
        n = s["name"]
        HW, nblk = s["HW"], s["nblk"]
        o = res[f"opack{n}"].reshape(128, B_LOC, nblk, 3, 6)
        rows = (o.transpose(1, 2, 0, 3, 4)
                .reshape(B_LOC, nblk * 128, 3, 6)[:, :HW]
                .reshape(B_LOC * HW * 3, 6))
        per_scale.append(rows)
    return per_scale


def kernel(output_13, output_26, output_52, anchors_13, anchors_26,
           anchors_52, threshold):
    from concourse.bass_utils import run_bass_kernel_spmd

    nc = _get_program()
    outs = (np.asarray(output_13), np.asarray(output_26),
            np.asarray(output_52))
    anchors = (np.asarray(anchors_13), np.asarray(anchors_26),
               np.asarray(anchors_52))
    thr = np.asarray(threshold)

    in_maps = [_core_inputs(cc, outs, anchors, thr) for cc in range(N_CORES)]
    r = run_bass_kernel_spmd(nc, in_maps, list(range(N_CORES)))
    per_core = [_assemble_core(r.results[cc]) for cc in range(N_CORES)]
    blocks = []
    for si in range(3):
        blocks.append(np.concatenate([per_core[cc][si]
                                      for cc in range(N_CORES)], axis=0))
    return np.concatenate(blocks, axis=0).astype(np.float32)


# revision 18
# speedup vs baseline: 3.3733x; 1.2714x over previous
"""YOLOv3-style detection decode kernel for Trainium2 (8 NeuronCores).

Data-parallel over batch (32 -> 4 per core). Host marshals each core's head
tensors into a cells-on-partitions layout x[p, (b k a c)] (cell = k*128+p,
c = 85 attrs per anchor); since 3 anchors * 85 = 255 = the channel count,
(b, k, a) collapse into one free dim Z and the device needs no transposes:

  - argmax over the 80 classes per (cell, anchor) via two segmented DVE
    reductions: phase-maxes p8[j] = max_g x[8g+j] and group-maxes
    q10[g] = max_j x[8g+j] (one tensor_reduce each per scale). The class
    index is 8*g* + j*, with g*/j* recovered by an is_ge-against-max
    compare and a descending-weight max (ties break toward the FIRST
    index, matching jnp.argmax).
  - box decode reads strided views of the same tiles (exp/scale on ACT).
  - outputs are packed [p, b, k, a, 6] per scale; the host re-interleaves.
"""

import sys

import numpy as np

if "/opt/trn_rl_repo" not in sys.path:
    sys.path.insert(0, "/opt/trn_rl_repo")

NUM_ATTRS = 85
B_LOC = 4  # batches per core (32 / 8)
N_CORES = 8

# (name, H, stride)
_SCALES = (
    ("13", 13, 32.0),
    ("26", 26, 16.0),
    ("52", 52, 8.0),
)


def _scale_cfg():
    cfgs = []
    for name, H, stride in _SCALES:
        HW = H * H
        nblk = -(-HW // 128)
        cfgs.append(dict(name=name, H=H, W=H, HW=HW, stride=stride,
                         nblk=nblk, HWp=nblk * 128))
    return cfgs


SCFG = _scale_cfg()

# consts layout: w8(8) | w10(10) | thr(1) | per scale: gx4(4nb) gy4(4nb) anch(6)
_CST_W8 = 0
_CST_W10 = 8
_CST_THR = 18
_CST_SC = {}
_off = 19
for _s in SCFG:
    _CST_SC[_s["name"]] = _off
    _off += 8 * _s["nblk"] + 6
CST_COLS = _off


def _build_program():
    import concourse.bass as bass
    import concourse.mybir as mybir
    from concourse.tile import TileContext

    f32 = mybir.dt.float32
    bf16 = mybir.dt.bfloat16
    Alu = mybir.AluOpType
    Act = mybir.ActivationFunctionType
    X = mybir.AxisListType.X

    nc = bass.Bass(trn_type="TRN2")

    xin = {}
    opk = {}
    for s in SCFG:
        n = s["name"]
        xin[n] = nc.declare_dram_parameter(
            f"x{n}", [128, B_LOC * s["nblk"] * 255], f32, False)
        opk[n] = nc.declare_dram_parameter(
            f"opack{n}", [128, B_LOC * s["nblk"] * 18], f32, True)
    cst_p = nc.declare_dram_parameter("cst", [128, CST_COLS], f32, False)

    with TileContext(nc) as tc:
        from contextlib import ExitStack
        with ExitStack() as ctx:
            cpool = ctx.enter_context(tc.tile_pool(name="consts", bufs=1))
            x52pool = ctx.enter_context(tc.tile_pool(name="x52", bufs=2))
            # fold scratch: consumed only by the (serial) DVE queue, so a
            # single buffer per tag costs no parallelism
            fpool = ctx.enter_context(tc.tile_pool(name="folds", bufs=1))

            cstt = cpool.tile([128, CST_COLS], f32, tag="cst", name="cstt")
            nc.sync.dma_start(out=cstt[:, :], in_=cst_p[:, :])
            thr = cstt[:, _CST_THR:_CST_THR + 1]

            # bf16 copies of the index weights (2-byte dtype enables the
            # DVE 2x mode for the small extraction ops)
            w8c = cpool.tile([128, 8], bf16, tag="w8c", name="w8c")
            nc.vector.tensor_copy(out=w8c[:, :],
                                  in_=cstt[:, _CST_W8:_CST_W8 + 8])
            w10c = cpool.tile([128, 10], bf16, tag="w10c", name="w10c")
            nc.vector.tensor_copy(out=w10c[:, :],
                                  in_=cstt[:, _CST_W10:_CST_W10 + 10])

            def scale_tiles(s):
                n, nb = s["name"], s["nblk"]
                Z = B_LOC * nb * 3
                t = {}
                for key, w in (("p8", 8), ("q10", 10), ("m", 1),
                               ("ts", 1), ("mask", 1), ("ex", 2), ("wh", 2),
                               ("cx", 1), ("cy", 1), ("x1", 1), ("y1", 1),
                               ("x2", 1), ("y2", 1), ("opk", 6)):
                    t[key] = cpool.tile([128, Z * w], f32, tag=f"{key}{n}",
                                        name=f"{key}{n}")
                for key, w in (("eq8", 8), ("eq10", 10), ("ew8", 8),
                               ("ew10", 10), ("r8", 1), ("r10", 1)):
                    t[key] = cpool.tile([128, Z * w], bf16, tag=f"{key}{n}",
                                        name=f"{key}{n}")
                return t

            def class_reduces(s, t, xtv, b):
                """Phase/group reduces + conf + cx/cy for one input view.

                xtv: [p, z, c=85] view (z spans (k a) or (b k a)); b is None
                for the all-batch case, else the batch index of a transient
                tile.
                """
                n, nb, stride = s["name"], s["nblk"], s["stride"]
                off = _CST_SC[n]
                zc = xtv.shape[1]  # cells*anchors covered by this view
                lo = 0 if b is None else b * nb * 3
                hi = lo + zc

                cls = xtv[:, :, 5:85]
                p8v = t["p8"][:, lo * 8:hi * 8].rearrange(
                    "p (z j) -> p z j", j=8)
                q10v = t["q10"][:, lo * 10:hi * 10].rearrange(
                    "p (z g) -> p z g", g=10)

                # Tournament folds (tensor_tensor reads two streams/cycle,
                # so folds cost half a single-stream tensor_reduce; all
                # slices are stride-1 in the inner dim).
                # phase-max p8[j] = max_g cls[8g+j]: fold the group axis.
                f1 = fpool.tile([128, zc * 40], f32, tag="f1", name="f1")
                f1v = f1[:, :].rearrange("p (z c) -> p z c", c=40)
                nc.vector.tensor_tensor(out=f1v, in0=cls[:, :, 0:40],
                                        in1=cls[:, :, 40:80], op=Alu.max)
                f2 = fpool.tile([128, zc * 16], f32, tag="f2", name="f2")
                f2v = f2[:, :].rearrange("p (z c) -> p z c", c=16)
                nc.vector.tensor_tensor(out=f2v, in0=f1v[:, :, 0:16],
                                        in1=f1v[:, :, 16:32], op=Alu.max)
                f3 = fpool.tile([128, zc * 8], f32, tag="f3", name="f3")
                f3v = f3[:, :].rearrange("p (z c) -> p z c", c=8)
                nc.vector.tensor_tensor(out=f3v, in0=f2v[:, :, 0:8],
                                        in1=f2v[:, :, 8:16], op=Alu.max)
                nc.vector.tensor_tensor(out=p8v, in0=f3v,
                                        in1=f1v[:, :, 32:40], op=Alu.max)
                # group-max q10[g] = max_j cls[8g+j]: fold within groups.
                cg = cls.rearrange("p z (g j) -> p z g j", g=10, j=8)
                y1 = fpool.tile([128, zc * 40], f32, tag="y1", name="y1")
                y1v = y1[:, :].rearrange("p (z g j) -> p z g j", g=10, j=4)
                nc.vector.tensor_tensor(out=y1v, in0=cg[:, :, :, 0:4],
                                        in1=cg[:, :, :, 4:8], op=Alu.max)
                y1z = y1[:, :].rearrange("p (w j) -> p w j", j=4)
                y2 = fpool.tile([128, zc * 20], f32, tag="y2", name="y2")
                y2z = y2[:, :].rearrange("p (w j) -> p w j", j=2)
                nc.vector.tensor_tensor(out=y2z, in0=y1z[:, :, 0:2],
                                        in1=y1z[:, :, 2:4], op=Alu.max)
                nc.vector.tensor_tensor(
                    out=t["q10"][:, lo * 10:hi * 10],
                    in0=y2z[:, :, 0:1].squeeze(2),
                    in1=y2z[:, :, 1:2].squeeze(2), op=Alu.max)

                conf = xtv[:, :, 0:1].squeeze(2)
                maskv = t["mask"][:, lo:hi]
                nc.vector.tensor_single_scalar(
                    out=maskv, in_=conf, scalar=thr, op=Alu.is_gt)
                opkz = t["opk"][:, lo * 6:hi * 6].rearrange(
                    "p (z q) -> p z q", q=6)
                nc.vector.tensor_tensor(
                    out=opkz[:, :, 0:1].squeeze(2), in0=conf, in1=maskv,
                    op=Alu.mult)

                # exp(tw,th) on ACT; cx/cy on DVE
                exv = t["ex"][:, lo * 2:hi * 2].rearrange(
                    "p (z e) -> p z e", e=2)
                nc.scalar.activation(out=exv, in_=xtv[:, :, 3:5],
                                     func=Act.Exp)
                # gx4/gy4 are host-replicated over batches, so a [p, zc/3, 3]
                # broadcast view always lines up with this z range.
                gx = cstt[:, off + lo // 3:off + hi // 3]
                gy = cstt[:, off + 4 * nb + lo // 3:off + 4 * nb + hi // 3]
                gxb = gx.unsqueeze(2).broadcast_to([128, zc // 3, 3])
                gyb = gy.unsqueeze(2).broadcast_to([128, zc // 3, 3])
                tx = xtv[:, :, 1:2].squeeze(2).rearrange(
                    "p (w a) -> p w a", a=3)
                ty = xtv[:, :, 2:3].squeeze(2).rearrange(
                    "p (w a) -> p w a", a=3)
                cxv = t["cx"][:, lo:hi].rearrange("p (w a) -> p w a", a=3)
                cyv = t["cy"][:, lo:hi].rearrange("p (w a) -> p w a", a=3)
                nc.vector.scalar_tensor_tensor(
                    out=cxv, in0=tx, scalar=stride, in1=gxb,
                    op0=Alu.mult, op1=Alu.add)
                nc.vector.scalar_tensor_tensor(
                    out=cyv, in0=ty, scalar=stride, in1=gyb,
                    op0=Alu.mult, op1=Alu.add)

            def per_scale(s, t):
                """All-batch epilogue on compact scratch tiles."""
                n, nb = s["name"], s["nblk"]
                off = _CST_SC[n]
                anch = cstt[:, off + 8 * nb:off + 8 * nb + 6]
                Z = B_LOC * nb * 3

                def zv(tile, w):
                    return tile[:, :].rearrange("p (z q) -> p z q", q=w)

                p8 = zv(t["p8"], 8)
                q10 = zv(t["q10"], 10)
                eq8 = zv(t["eq8"], 8)
                eq10 = zv(t["eq10"], 10)
                ew8 = zv(t["ew8"], 8)
                ew10 = zv(t["ew10"], 10)
                m = t["m"][:, :]
                r8 = t["r8"][:, :]
                r10 = t["r10"][:, :]
                tsv = t["ts"][:, :]
                mask = t["mask"][:, :]
                opkq = zv(t["opk"], 6)

                nc.vector.tensor_reduce(out=m, in_=p8, axis=X, op=Alu.max)
                mb8 = m.unsqueeze(2).broadcast_to([128, Z, 8])
                mb10 = m.unsqueeze(2).broadcast_to([128, Z, 10])
                w8b = w8c[:, :].unsqueeze(1).broadcast_to([128, Z, 8])
                w10b = w10c[:, :].unsqueeze(1).broadcast_to([128, Z, 10])

                # j* / g* via descending-weight max (first-index tie-break);
                # eq/weights/r are exact small values -> bf16 (2x DVE mode)
                nc.vector.tensor_tensor(out=eq8, in0=p8, in1=mb8,
                                        op=Alu.is_ge)
                nc.vector.tensor_tensor(out=ew8, in0=eq8, in1=w8b,
                                        op=Alu.mult)
                nc.vector.tensor_reduce(out=r8, in_=ew8, axis=X, op=Alu.max)
                nc.vector.tensor_tensor(out=eq10, in0=q10, in1=mb10,
                                        op=Alu.is_ge)
                nc.vector.tensor_tensor(out=ew10, in0=eq10, in1=w10b,
                                        op=Alu.mult)
                nc.vector.tensor_reduce(out=r10, in_=ew10, axis=X,
                                        op=Alu.max)
                # idx = 88 - 8*r10 - r8 ; cls_m = (ts + 88) * mask
                nc.vector.scalar_tensor_tensor(
                    out=tsv, in0=r10, scalar=-8.0, in1=r8,
                    op0=Alu.mult, op1=Alu.subtract)
                nc.vector.scalar_tensor_tensor(
                    out=opkq[:, :, 5:6].squeeze(2), in0=tsv, scalar=88.0,
                    in1=mask, op0=Alu.add, op1=Alu.mult)

                # boxes: wh = anch * exp (ACT) -> x1/y1/x2/y2 -> masked
                exw = t["ex"][:, :].rearrange("p (w a e) -> p w a e",
                                              a=3, e=2)
                whw = t["wh"][:, :].rearrange("p (w a e) -> p w a e",
                                              a=3, e=2)
                for a in range(3):
                    for q in range(2):
                        nc.scalar.activation(
                            out=whw[:, :, a, q:q + 1].squeeze(2),
                            in_=exw[:, :, a, q:q + 1].squeeze(2),
                            func=Act.Copy,
                            scale=anch[:, 2 * a + q:2 * a + q + 1])
                whz = zv(t["wh"], 2)
                wx = whz[:, :, 0:1].squeeze(2)
                wy = whz[:, :, 1:2].squeeze(2)
                cx = t["cx"][:, :]
                cy = t["cy"][:, :]
                x1 = t["x1"][:, :]
                y1 = t["y1"][:, :]
                x2 = t["x2"][:, :]
                y2 = t["y2"][:, :]
                nc.vector.scalar_tensor_tensor(
                    out=x1, in0=wx, scalar=-0.5, in1=cx,
                    op0=Alu.mult, op1=Alu.add)
                nc.vector.scalar_tensor_tensor(
                    out=y1, in0=wy, scalar=-0.5, in1=cy,
                    op0=Alu.mult, op1=Alu.add)
                nc.vector.scalar_tensor_tensor(
                    out=x2, in0=wx, scalar=0.5, in1=cx,
                    op0=Alu.mult, op1=Alu.add)
                nc.vector.scalar_tensor_tensor(
                    out=y2, in0=wy, scalar=0.5, in1=cy,
                    op0=Alu.mult, op1=Alu.add)
                for q, src in ((1, x1), (2, y1), (3, x2), (4, y2)):
                    nc.vector.tensor_tensor(
                        out=opkq[:, :, q:q + 1].squeeze(2), in0=src,
                        in1=mask, op=Alu.mult)
                nc.sync.dma_start(out=opk[n][:, :], in_=t["opk"][:, :])

            # ---- scales 13 / 26: one static tile, all batches at once ----
            for s in SCFG[:2]:
                n = s["name"]
                t = scale_tiles(s)
                xt = cpool.tile([128, B_LOC * s["nblk"] * 255], f32,
                                tag=f"xt{n}", name=f"xt{n}")
                nc.sync.dma_start(out=xt[:, :], in_=xin[n][:, :])
                vz = xt[:, :].rearrange("p (z c) -> p z c", c=85)
                class_reduces(s, t, vz, None)
                per_scale(s, t)

            # ---- scale 52: per-batch pipelined input tiles ----
            s = SCFG[2]
            t = scale_tiles(s)
            nb = s["nblk"]
            for b in range(B_LOC):
                xtb = x52pool.tile([128, nb * 255], f32, tag="x52",
                                   name="x52b")
                nc.sync.dma_start(
                    out=xtb[:, :],
                    in_=xin[s["name"]][:, b * nb * 255:(b + 1) * nb * 255])
                vz = xtb[:, :].rearrange("p (z c) -> p z c", c=85)
                class_reduces(s, t, vz, b)
            per_scale(s, t)

    return nc


def _split_sync_waits(nc, limit=1):
    """Move overflow sync waits onto standalone NoOps (several instruction
    structs only have one wait slot; walrus hard-errors otherwise)."""
    import concourse.mybir as mybir

    for f in nc.m.functions:
        for blk in f.blocks:
            out = []
            changed = False
            for i in blk.instructions:
                si = i.sync_info
                tname = type(i).__name__
                if (si is not None and si.on_wait
                        and len(si.on_wait) > limit
                        and tname not in ("InstEventSemaphore",)):
                    waits = list(si.on_wait)
                    keep = waits[-limit:]
                    spill = waits[:-limit]
                    for k, w in enumerate(spill):
                        nop = mybir.InstNoOp(
                            name=f"{i.name}-sw{k}", ins=[], outs=[])
                        nop.engine = i.engine
                        nop.sync_info = mybir.SyncInfo(
                            on_wait=[w], on_update=[])
                        out.append(nop)
                    i.sync_info = mybir.SyncInfo(
                        on_wait=keep, on_update=list(si.on_update or []))
                    changed = True
                out.append(i)
            if changed:
                blk.instructions = out


_NC_CACHE = None


def _get_program(split=True):
    global _NC_CACHE
    if _NC_CACHE is None:
        _NC_CACHE = _build_program()
    if split and not getattr(_NC_CACHE, "_waits_split", False):
        _split_sync_waits(_NC_CACHE)
        _NC_CACHE._waits_split = True
    return _NC_CACHE


def _core_inputs(core, outs, anchors, threshold):
    """Build the DRAM input map for one core. Pure data marshaling."""
    m = {}
    for s, x_full in zip(SCFG, outs):
        n = s["name"]
        HW, nblk = s["HW"], s["nblk"]
        x = np.asarray(
            x_full[core * B_LOC:(core + 1) * B_LOC], dtype=np.float32
        ).reshape(B_LOC, 255, HW)
        xp = np.zeros((B_LOC, 255, nblk * 128), np.float32)
        xp[:, :, :HW] = x
        # [b, c, k, p] -> [p, b, k, c]
        m[f"x{n}"] = np.ascontiguousarray(
            xp.reshape(B_LOC, 255, nblk, 128).transpose(3, 0, 2, 1)
        ).reshape(128, -1)
    cst = np.zeros((128, CST_COLS), np.float32)
    cst[:, _CST_W8:_CST_W8 + 8] = (8.0 - np.arange(8))[None, :]
    cst[:, _CST_W10:_CST_W10 + 10] = (10.0 - np.arange(10))[None, :]
    cst[:, _CST_THR] = np.float32(np.asarray(threshold)[0])
    for s, anch in zip(SCFG, anchors):
        n = s["name"]
        HW, nblk, W, stride = s["HW"], s["nblk"], s["W"], s["stride"]
        off = _CST_SC[n]
        cell = (np.arange(nblk)[None, :] * 128
                + np.arange(128)[:, None])  # [p, k]
        valid = cell < HW
        gx = np.where(valid, (cell % W) * stride, 0.0).astype(np.float32)
        gy = np.where(valid, (cell // W) * stride, 0.0).astype(np.float32)
        cst[:, off:off + 4 * nblk] = np.tile(gx, (1, B_LOC))
        cst[:, off + 4 * nblk:off + 8 * nblk] = np.tile(gy, (1, B_LOC))
        cst[:, off + 8 * nblk:off + 8 * nblk + 6] = np.asarray(
            anch, np.float32).reshape(6)[None, :]
    m["cst"] = cst
    return m


def _assemble_core(res):
    """Interleave one core's packed outputs into reference row order."""
    per_scale = []
    for s in SCFG:
        n = s["name"]
        HW, nblk = s["HW"], s["nblk"]
        o = res[f"opack{n}"].reshape(128, B_LOC, nblk, 3, 6)
        rows = (o.transpose(1, 2, 0, 3, 4)
                .reshape(B_LOC, nblk * 128, 3, 6)[:, :HW]
                .reshape(B_LOC * HW * 3, 6))
        per_scale.append(rows)
    return per_scale


def kernel(output_13, output_26, output_52, anchors_13, anchors_26,
           anchors_52, threshold):
    from concourse.bass_utils import run_bass_kernel_spmd

    nc = _get_program()
    outs = (np.asarray(output_13), np.asarray(output_26),
            np.asarray(output_52))
    anchors = (np.asarray(anchors_13), np.asarray(anchors_26),
               np.asarray(anchors_52))
    thr = np.asarray(threshold)

    in_maps = [_core_inputs(cc, outs, anchors, thr) for cc in range(N_CORES)]
    r = run_bass_kernel_spmd(nc, in_maps, list(range(N_CORES)))
    per_core = [_assemble_core(r.results[cc]) for cc in range(N_CORES)]
    blocks = []
    for si in range(3):
        blocks.append(np.concatenate([per_core[cc][si]
                                      for cc in range(N_CORES)], axis=0))
    return np.concatenate(blocks, axis=0).astype(np.float32)
